# revision 1
# baseline (speedup 1.0000x reference)
"""Causal GQA self-attention (B=2,T=2048,C=4096, 32 q-heads, 8 kv-groups, hs=128)
sharded tensor-parallel across 8 TRN2 NeuronCores: one kv-group (4 q heads) per core.

Per-core pipeline (all activations feature-major, i.e. transposed):
  1. qkvT = Wqkv_g^T @ x^T           (bf16 matmul, fp32 psum)
     RoPE applied to q/k in fp32 during psum->sbuf epilogue, cast bf16
  2. v transposed to token-major via PE transpose
  3. per (batch, head): S^T = k'T^T-slices @ q'T  (scores transposed:
     kv-pos on partitions, q-pos on free dim), exp via ACT (scale folded),
     causal mask by multiply on diagonal tiles, O^T accumulated via
     v_tok^T @ P^T, denominator via ones^T @ P^T
  4. normalization: 1/denom via ACT exp(-ln(d)), broadcast across
     partitions via PE outer-product, multiply into O^T
  5. partial out = y^T-slices^T @ Wproj_g   -> [4096 tok, 4096] bf16
Host sums the 8 partial outputs in fp32.
"""
import math

import numpy as np
import ml_dtypes

import concourse.bass as bass
import concourse.mybir as mybir
import concourse.tile as tile
from concourse import bacc
from concourse.bass_utils import run_bass_kernel_spmd

BF16 = mybir.dt.bfloat16
F32 = mybir.dt.float32
AF = mybir.ActivationFunctionType

N_CORES = 8
B, T, C = 2, 2048, 4096
HS = 128
QPK = 4              # q heads per kv group
GCOLS = (QPK + 2) * HS   # 768 qkv columns per group
TOK = B * T          # 4096
SCALE = float(1.0 / np.sqrt(np.float32(HS)))

_NC_CACHE = None


def build_nc():
    nc = bacc.Bacc("TRN2", target_bir_lowering=False, debug=False,
                   num_devices=N_CORES)
    xT = nc.dram_tensor("xt", [C, TOK], BF16, kind="ExternalInput").ap()
    wqkv = nc.dram_tensor("wqkv", [C, GCOLS], BF16, kind="ExternalInput").ap()
    wproj = nc.dram_tensor("wproj", [QPK * HS, C], BF16, kind="ExternalInput").ap()
    cosf = nc.dram_tensor("cosf", [128, T], F32, kind="ExternalInput").ap()
    sinb = nc.dram_tensor("sinb", [128, T], F32, kind="ExternalInput").ap()
    masks = nc.dram_tensor("masks", [128, 4, 512], BF16, kind="ExternalInput").ap()
    onesc = nc.dram_tensor("onesc", [128, 1], BF16, kind="ExternalInput").ap()
    onesr = nc.dram_tensor("onesr", [1, 128], F32, kind="ExternalInput").ap()
    ident = nc.dram_tensor("ident", [128, 128], BF16, kind="ExternalInput").ap()
    out = nc.dram_tensor("out", [TOK, C], BF16, kind="ExternalOutput").ap()

    xT_r = xT.rearrange("(ko p) t -> p ko t", p=128)        # [128, 32, TOK]
    wqkv_r = wqkv.rearrange("(ko p) m -> p ko m", p=128)    # [128, 32, 768]
    wproj_r = wproj.rearrange("(h p) c -> p h c", p=128)    # [128, 4, C]

    with tile.TileContext(nc) as tc:
        with tc.tile_pool(name="mid", bufs=1) as mid:
            qT = mid.tile([128, QPK, TOK], BF16)   # rope'd q, feature-major
            kT = mid.tile([128, TOK], BF16)
            vT = mid.tile([128, TOK], BF16)

            # ---------------- stage 1: qkv + rope ----------------
            with tc.tile_pool(name="s1c", bufs=1) as s1c, \
                 tc.tile_pool(name="s1x", bufs=4) as s1x, \
                 tc.tile_pool(name="s1r", bufs=3) as s1r, \
                 tc.tile_pool(name="ps1", bufs=4, space="PSUM") as ps1:
                wq_sb = s1c.tile([128, 32, GCOLS], BF16)
                for kk in range(32):
                    nc.sync.dma_start(wq_sb[:, kk, :], wqkv_r[:, kk, :])
                cos_sb = s1c.tile([128, T], F32)
                sin_sb = s1c.tile([128, T], F32)

                for tcch in range(TOK // 512):       # 8 chunks of 512 tokens
                    t0 = tcch * 512
                    tb = t0 % T                      # position within batch
                    xa = s1x.tile([128, 16, 512], BF16, tag="x")
                    for q4 in range(4):
                        nc.sync.dma_start(
                            xa[:, q4 * 4:(q4 + 1) * 4, :],
                            xT_r[:, q4 * 4:(q4 + 1) * 4, t0:t0 + 512])
                    xb = s1x.tile([128, 16, 512], BF16, tag="x")
                    for q4 in range(4):
                        nc.sync.dma_start(
                            xb[:, q4 * 4:(q4 + 1) * 4, :],
                            xT_r[:, 16 + q4 * 4:16 + (q4 + 1) * 4, t0:t0 + 512])
                    if tcch == 0:
                        nc.sync.dma_start(cos_sb[:], cosf[:])
                        nc.sync.dma_start(sin_sb[:], sinb[:])
                    for m in range(6):
                        ps = ps1.tile([128, 512], F32, tag="ps")
                        for kh, xt_t in ((0, xa), (1, xb)):
                            for ki in range(16):
                                kk = kh * 16 + ki
                                nc.tensor.matmul(
                                    ps[:], wq_sb[:, kk, m * 128:(m + 1) * 128],
                                    xt_t[:, ki, :],
                                    start=(kk == 0), stop=(kk == 31))
                        if m == 5:                   # v: plain copy+cast
                            nc.vector.tensor_copy(vT[:, t0:t0 + 512], ps[:])
                        else:                        # q heads 0-3 / k: rope
                            t1 = s1r.tile([128, 512], F32, tag="t1")
                            nc.vector.tensor_mul(t1[:], ps[:],
                                                 cos_sb[:, tb:tb + 512])
                            u = s1r.tile([128, 512], F32, tag="u")
                            nc.vector.tensor_mul(u[0:64, :], ps[64:128, :],
                                                 sin_sb[0:64, tb:tb + 512])
                            nc.vector.tensor_mul(u[64:128, :], ps[0:64, :],
                                                 sin_sb[64:128, tb:tb + 512])
                            if m < 4:
                                dst = qT[:, m, t0:t0 + 512]
                            else:
                                dst = kT[:, t0:t0 + 512]
                            nc.vector.tensor_add(dst, t1[:], u[:])

            # ---------------- stages 2+3: attention ----------------
            with tc.tile_pool(name="s3c", bufs=1) as s3c, \
                 tc.tile_pool(name="s3p", bufs=8) as s3p:
                ident_sb = s3c.tile([128, 128], BF16)
                nc.sync.dma_start(ident_sb[:], ident[:])
                onesc_sb = s3c.tile([128, 1], BF16)
                nc.sync.dma_start(onesc_sb[:], onesc[:])
                onesr_sb = s3c.tile([1, 128], F32)
                nc.sync.dma_start(onesr_sb[:], onesr[:])
                masks_sb = s3c.tile([128, 4, 512], BF16)
                nc.sync.dma_start(masks_sb[:], masks[:])
                v_tok = s3c.tile([128, 32, 128], BF16)
                yT = s3c.tile([128, QPK, TOK], BF16)
                wp_sb = s3c.tile([128, 4, C], BF16)
                for hh in range(4):
                    nc.sync.dma_start(wp_sb[:, hh, :], wproj_r[:, hh, :])

                with tc.tile_pool(name="psT", bufs=4, space="PSUM") as psT:
                    # v -> token-major via PE transpose
                    for si in range(32):
                        pt_ps = psT.tile([128, 128], BF16, tag="tp")
                        nc.tensor.transpose(pt_ps[:],
                                            vT[:, si * 128:(si + 1) * 128],
                                            ident_sb[:])
                        nc.vector.tensor_copy(v_tok[:, si, :], pt_ps[:])

                psum_ctx = [tc.tile_pool(name="pss", bufs=3, space="PSUM"),
                            tc.tile_pool(name="pso", bufs=2, space="PSUM"),
                            tc.tile_pool(name="psd", bufs=2, space="PSUM"),
                            tc.tile_pool(name="psb", bufs=1, space="PSUM")]
                pss, pso, psd, psb = [c.__enter__() for c in psum_ctx]
                rtp_cm = tc.tile_pool(name="rtp", bufs=4)
                rtp = rtp_cm.__enter__()
                psum_ctx.append(rtp_cm)

                for b in range(B):
                    for tcq in range(4):              # 512-token q chunks
                        for h in range(QPK):
                            t0g = b * T + tcq * 512
                            n_s = (tcq + 1) * 4
                            ps_o = pso.tile([128, 512], F32, tag="o")
                            ps_d = psd.tile([1, 512], F32, tag="d")
                            for si in range(n_s):
                                s0g = b * T + si * 128
                                ps_s = pss.tile([128, 512], F32, tag="s")
                                nc.tensor.matmul(
                                    ps_s[:], kT[:, s0g:s0g + 128],
                                    qT[:, h, t0g:t0g + 512],
                                    start=True, stop=True)
                                pt = s3p.tile([128, 512], BF16, tag="pt")
                                nc.scalar.activation(pt[:], ps_s[:], AF.Exp,
                                                     scale=SCALE)
                                if si >= tcq * 4:
                                    j = si - tcq * 4
                                    nc.vector.tensor_mul(pt[:], pt[:],
                                                         masks_sb[:, j, :])
                                nc.tensor.matmul(
                                    ps_o[:], v_tok[:, b * 16 + si, :], pt[:],
                                    start=(si == 0), stop=(si == n_s - 1))
                                nc.tensor.matmul(
                                    ps_d[:], onesc_sb[:], pt[:],
                                    start=(si == 0), stop=(si == n_s - 1))
                            rden = rtp.tile([1, 512], F32, tag="rd")
                            nc.vector.reciprocal_approx_fast(rden[:], ps_d[:])
                            ps_bc = psb.tile([128, 512], F32, tag="bc")
                            nc.tensor.matmul(ps_bc[:], onesr_sb[:], rden[:],
                                             start=True, stop=True)
                            rb = rtp.tile([128, 512], F32, tag="rb")
                            nc.vector.tensor_copy(rb[:], ps_bc[:])
                            nc.vector.tensor_mul(yT[:, h, t0g:t0g + 512],
                                                 ps_o[:], rb[:])

                for c in reversed(psum_ctx):
                    c.__exit__(None, None, None)

                # ---------------- stage 4: proj ----------------
                with tc.tile_pool(name="s4o", bufs=4) as s4o, \
                     tc.tile_pool(name="ps4", bufs=4, space="PSUM") as ps4:
                    for ti in range(TOK // 128):
                        t0 = ti * 128
                        for cc in range(C // 512):
                            ps_p = ps4.tile([128, 512], F32, tag="p")
                            for h in range(QPK):
                                nc.tensor.matmul(
                                    ps_p[:], yT[:, h, t0:t0 + 128],
                                    wp_sb[:, h, cc * 512:(cc + 1) * 512],
                                    start=(h == 0), stop=(h == 3))
                            ob = s4o.tile([128, 512], BF16, tag="ob")
                            nc.vector.tensor_copy(ob[:], ps_p[:])
                            nc.sync.dma_start(
                                out[t0:t0 + 128, cc * 512:(cc + 1) * 512],
                                ob[:])
    nc.compile()
    return nc


def _prep_inputs(x, cos, sin, Wqkv, Wproj):
    bf = ml_dtypes.bfloat16
    xTn = np.ascontiguousarray(x.reshape(TOK, C).T).astype(bf)
    cosf = np.ascontiguousarray(cos.T.astype(np.float32))        # [128, T]
    sinT = cos_s = sin.T.astype(np.float32)
    sinb = np.concatenate([-sinT[0:64], sinT[64:128]], axis=0)
    sinb = np.ascontiguousarray(sinb, dtype=np.float32)
    p = np.arange(128)[:, None, None]
    j = np.arange(4)[None, :, None]
    f = np.arange(512)[None, None, :]
    masks = ((j * 128 + p) <= f).astype(bf)
    onesc = np.ones([128, 1], dtype=bf)
    onesr = np.ones([1, 128], dtype=np.float32)
    ident = np.eye(128, dtype=np.float32).astype(bf)
    in_maps = []
    for g in range(N_CORES):
        in_maps.append({
            "xt": xTn,
            "wqkv": np.ascontiguousarray(Wqkv[:, g * GCOLS:(g + 1) * GCOLS]).astype(bf),
            "wproj": np.ascontiguousarray(Wproj[g * 512:(g + 1) * 512, :]).astype(bf),
            "cosf": cosf, "sinb": sinb, "masks": masks,
            "onesc": onesc, "onesr": onesr, "ident": ident,
        })
    return in_maps


def kernel(x, cos, sin, Wqkv, Wproj, _trace=False):
    global _NC_CACHE
    x = np.asarray(x, dtype=np.float32)
    cos = np.asarray(cos, dtype=np.float32)
    sin = np.asarray(sin, dtype=np.float32)
    Wqkv = np.asarray(Wqkv, dtype=np.float32)
    Wproj = np.asarray(Wproj, dtype=np.float32)
    if _NC_CACHE is None:
        _NC_CACHE = build_nc()
    nc = _NC_CACHE
    in_maps = _prep_inputs(x, cos, sin, Wqkv, Wproj)
    res = run_bass_kernel_spmd(nc, in_maps, core_ids=list(range(N_CORES)),
                               trace=_trace)
    acc = np.zeros([TOK, C], dtype=np.float32)
    for r in res.results:
        acc += r["out"].astype(np.float32)
    if _trace:
        kernel._last_exec_ns = res.exec_time_ns
        kernel._last_trace = res.instructions_and_trace
    return acc.reshape(B, T, C)



# revision 3
# speedup vs baseline: 1.1288x; 1.1288x over previous
"""Causal GQA self-attention (B=2,T=2048,C=4096, 32 q-heads, 8 kv-groups, hs=128)
sharded tensor-parallel across 8 TRN2 NeuronCores: one kv-group (4 q heads) per core.

Per-core pipeline (all activations feature-major, i.e. transposed):
  1. qkvT = Wqkv_g^T @ x^T           (bf16 matmul, fp32 psum)
     RoPE applied to q/k in fp32 during psum->sbuf epilogue, cast bf16
  2. v transposed to token-major via PE transpose
  3. attention per (batch, q-chunk, head-pair): S^T tiles (kv on partitions,
     q on free dim), exp via ACT with diagonal-trimmed columns, causal mask
     by multiply on diagonal tiles, O^T accumulated via v_tok^T @ P^T.
     Softmax denominator: exp tiles accumulated on DVE into dsum, one
     M=1 matmul per head contracts partitions; reciprocal + PE outer-product
     broadcast (bf16) + multiply into O^T.
     Head pairs share PE weight loads (same kT / v_tok stationary operand).
  4. proj tiles (y^T slices^T @ Wproj_g) interleaved into the attention
     stream as PE filler, so exp latency hides under proj matmuls.
Host sums the 8 partial outputs in fp32.
"""
import math

import numpy as np
import ml_dtypes

import concourse.bass as bass
import concourse.mybir as mybir
import concourse.tile as tile
from concourse import bacc
from concourse.bass_utils import run_bass_kernel_spmd

BF16 = mybir.dt.bfloat16
F32 = mybir.dt.float32
AF = mybir.ActivationFunctionType

N_CORES = 8
B, T, C = 2, 2048, 4096
HS = 128
QPK = 4              # q heads per kv group
GCOLS = (QPK + 2) * HS   # 768 qkv columns per group
TOK = B * T          # 4096
SCALE = float(1.0 / np.sqrt(np.float32(HS)))

_NC_CACHE = None


def build_nc():
    nc = bacc.Bacc("TRN2", target_bir_lowering=False, debug=False,
                   num_devices=N_CORES)
    xT = nc.dram_tensor("xt", [C, TOK], BF16, kind="ExternalInput").ap()
    wqkv = nc.dram_tensor("wqkv", [C, GCOLS], BF16, kind="ExternalInput").ap()
    wproj = nc.dram_tensor("wproj", [QPK * HS, C], BF16, kind="ExternalInput").ap()
    cosf = nc.dram_tensor("cosf", [128, T], F32, kind="ExternalInput").ap()
    sinb = nc.dram_tensor("sinb", [128, T], F32, kind="ExternalInput").ap()
    masks = nc.dram_tensor("masks", [128, 4, 512], BF16, kind="ExternalInput").ap()
    onesc = nc.dram_tensor("onesc", [128, 1], BF16, kind="ExternalInput").ap()
    onesr = nc.dram_tensor("onesr", [1, 128], BF16, kind="ExternalInput").ap()
    ident = nc.dram_tensor("ident", [128, 128], BF16, kind="ExternalInput").ap()
    out = nc.dram_tensor("out", [TOK, C], BF16, kind="ExternalOutput").ap()

    xT_r = xT.rearrange("(ko p) t -> p ko t", p=128)        # [128, 32, TOK]
    wqkv_r = wqkv.rearrange("(ko p) m -> p ko m", p=128)    # [128, 32, 768]
    wproj_r = wproj.rearrange("(h p) c -> p h c", p=128)    # [128, 4, C]

    with tile.TileContext(nc) as tc:
        with tc.tile_pool(name="mid", bufs=1) as mid:
            qT = mid.tile([128, QPK, TOK], BF16)   # rope'd q, feature-major
            kT = mid.tile([128, TOK], BF16)
            vT = mid.tile([128, TOK], BF16)

            # ---------------- stage 1: qkv + rope ----------------
            with tc.tile_pool(name="s1c", bufs=1) as s1c, \
                 tc.tile_pool(name="s1x", bufs=4) as s1x, \
                 tc.tile_pool(name="s1r", bufs=3) as s1r, \
                 tc.tile_pool(name="ps1", bufs=4, space="PSUM") as ps1:
                wq_sb = s1c.tile([128, 32, GCOLS], BF16)
                for kk in range(32):
                    nc.sync.dma_start(wq_sb[:, kk, :], wqkv_r[:, kk, :])
                cos_sb = s1c.tile([128, T], F32)
                sin_sb = s1c.tile([128, T], F32)

                for tcch in range(TOK // 512):       # 8 chunks of 512 tokens
                    t0 = tcch * 512
                    tb = t0 % T                      # position within batch
                    xa = s1x.tile([128, 16, 512], BF16, tag="x")
                    for q4 in range(4):
                        nc.sync.dma_start(
                            xa[:, q4 * 4:(q4 + 1) * 4, :],
                            xT_r[:, q4 * 4:(q4 + 1) * 4, t0:t0 + 512])
                    xb = s1x.tile([128, 16, 512], BF16, tag="x")
                    for q4 in range(4):
                        nc.sync.dma_start(
                            xb[:, q4 * 4:(q4 + 1) * 4, :],
                            xT_r[:, 16 + q4 * 4:16 + (q4 + 1) * 4, t0:t0 + 512])
                    if tcch == 0:
                        nc.sync.dma_start(cos_sb[:], cosf[:])
                        nc.sync.dma_start(sin_sb[:], sinb[:])
                    for m in range(6):
                        ps = ps1.tile([128, 512], F32, tag="ps")
                        for kh, xt_t in ((0, xa), (1, xb)):
                            for ki in range(16):
                                kk = kh * 16 + ki
                                nc.tensor.matmul(
                                    ps[:], wq_sb[:, kk, m * 128:(m + 1) * 128],
                                    xt_t[:, ki, :],
                                    start=(kk == 0), stop=(kk == 31))
                        if m == 5:                   # v: plain copy+cast
                            nc.vector.tensor_copy(vT[:, t0:t0 + 512], ps[:])
                        else:                        # q heads 0-3 / k: rope
                            t1 = s1r.tile([128, 512], F32, tag="t1")
                            nc.vector.tensor_mul(t1[:], ps[:],
                                                 cos_sb[:, tb:tb + 512])
                            u = s1r.tile([128, 512], F32, tag="u")
                            nc.vector.tensor_mul(u[0:64, :], ps[64:128, :],
                                                 sin_sb[0:64, tb:tb + 512])
                            nc.vector.tensor_mul(u[64:128, :], ps[0:64, :],
                                                 sin_sb[64:128, tb:tb + 512])
                            if m < 4:
                                dst = qT[:, m, t0:t0 + 512]
                            else:
                                dst = kT[:, t0:t0 + 512]
                            nc.vector.tensor_add(dst, t1[:], u[:])

            # ---------------- stages 2-4: attention + proj ----------------
            with tc.tile_pool(name="s3c", bufs=1) as s3c, \
                 tc.tile_pool(name="s3p", bufs=8) as s3p, \
                 tc.tile_pool(name="s3d", bufs=2) as s3d, \
                 tc.tile_pool(name="rtp", bufs=2) as rtp, \
                 tc.tile_pool(name="s4o", bufs=4) as s4o:
                ident_sb = s3c.tile([128, 128], BF16)
                nc.sync.dma_start(ident_sb[:], ident[:])
                onesc_sb = s3c.tile([128, 1], BF16)
                nc.sync.dma_start(onesc_sb[:], onesc[:])
                onesr_sb = s3c.tile([1, 128], BF16)
                nc.sync.dma_start(onesr_sb[:], onesr[:])
                masks_sb = s3c.tile([128, 4, 512], BF16)
                nc.sync.dma_start(masks_sb[:], masks[:])
                v_tok = s3c.tile([128, 32, 128], BF16)
                yT = s3c.tile([128, QPK, TOK], BF16)
                wp_sb = s3c.tile([128, 4, C], BF16)
                for hh in range(4):
                    nc.sync.dma_start(wp_sb[:, hh, :], wproj_r[:, hh, :])

                with tc.tile_pool(name="psT", bufs=4, space="PSUM") as psT:
                    # v -> token-major via PE transpose
                    for si in range(32):
                        pt_ps = psT.tile([128, 128], BF16, tag="tp")
                        nc.tensor.transpose(pt_ps[:],
                                            vT[:, si * 128:(si + 1) * 128],
                                            ident_sb[:])
                        nc.vector.tensor_copy(v_tok[:, si, :], pt_ps[:])

                psum_ctx = [tc.tile_pool(name="pax", bufs=3, space="PSUM"),
                            tc.tile_pool(name="pso", bufs=3, space="PSUM"),
                            tc.tile_pool(name="ps4", bufs=2, space="PSUM")]
                pax, pso, ps4 = [c.__enter__() for c in psum_ctx]

                # proj work queue: tiles become available per (b, tcq) unit
                proj_queue = []     # (t0 row, cc) pairs ready to emit
                proj_state = {"n": 0}

                def emit_proj(k):
                    for _ in range(k):
                        if not proj_queue:
                            return
                        pt0r, cc = proj_queue.pop(0)
                        ps_p = ps4.tile([128, 512], F32, tag="p")
                        for h in range(QPK):
                            nc.tensor.matmul(
                                ps_p[:], yT[:, h, pt0r:pt0r + 128],
                                wp_sb[:, h, cc * 512:(cc + 1) * 512],
                                start=(h == 0), stop=(h == 3))
                        ob = s4o.tile([128, 512], BF16, tag="ob")
                        if proj_state["n"] % 2 == 0:
                            nc.vector.tensor_copy(ob[:], ps_p[:])
                        else:
                            nc.scalar.copy(ob[:], ps_p[:])
                        proj_state["n"] += 1
                        nc.sync.dma_start(
                            out[pt0r:pt0r + 128, cc * 512:(cc + 1) * 512],
                            ob[:])

                # zero-init the pt rotation buffers once: diagonal-trimmed exp
                # leaves a column prefix unwritten that must multiply to 0
                for _ in range(8):
                    ptz = s3p.tile([128, 512], BF16, tag="pt")
                    nc.vector.memset(ptz[:], 0.0)

                for b in range(B):
                    for tcq in range(4):              # 512-token q chunks
                        t0g = b * T + tcq * 512
                        n_s = (tcq + 1) * 4
                        for hp in range(2):           # head pairs
                            h0, h1 = 2 * hp, 2 * hp + 1
                            ps_o0 = pso.tile([128, 512], F32, tag="o")
                            ps_o1 = pso.tile([128, 512], F32, tag="o")
                            d0 = d1 = None
                            for si in range(n_s):
                                s0g = b * T + si * 128
                                j = si - 4 * tcq
                                lo = 128 * max(j, 0)   # diagonal column trim
                                ps_s0 = pax.tile([128, 512], F32, tag="x")
                                nc.tensor.matmul(
                                    ps_s0[:, lo:], kT[:, s0g:s0g + 128],
                                    qT[:, h0, t0g + lo:t0g + 512],
                                    start=True, stop=True)
                                ps_s1 = pax.tile([128, 512], F32, tag="x")
                                nc.tensor.matmul(
                                    ps_s1[:, lo:], kT[:, s0g:s0g + 128],
                                    qT[:, h1, t0g + lo:t0g + 512],
                                    start=True, stop=True)
                                emit_proj(1)
                                pt0 = s3p.tile([128, 512], BF16, tag="pt")
                                nc.scalar.activation(pt0[:, lo:], ps_s0[:, lo:],
                                                     AF.Exp, scale=SCALE)
                                pt1 = s3p.tile([128, 512], BF16, tag="pt")
                                nc.scalar.activation(pt1[:, lo:], ps_s1[:, lo:],
                                                     AF.Exp, scale=SCALE)
                                if j >= 0:
                                    nc.vector.tensor_mul(pt0[:], pt0[:],
                                                         masks_sb[:, j, :])
                                    nc.vector.tensor_mul(pt1[:], pt1[:],
                                                         masks_sb[:, j, :])
                                nc.tensor.matmul(
                                    ps_o0[:], v_tok[:, b * 16 + si, :], pt0[:],
                                    start=(si == 0), stop=(si == n_s - 1))
                                nc.tensor.matmul(
                                    ps_o1[:], v_tok[:, b * 16 + si, :], pt1[:],
                                    start=(si == 0), stop=(si == n_s - 1))
                                if si == 0:
                                    d0 = s3d.tile([128, 512], BF16, tag="d0")
                                    nc.vector.tensor_copy(d0[:], pt0[:])
                                    d1 = s3d.tile([128, 512], BF16, tag="d1")
                                    nc.vector.tensor_copy(d1[:], pt1[:])
                                else:
                                    nc.vector.tensor_add(d0[:], d0[:], pt0[:])
                                    nc.vector.tensor_add(d1[:], d1[:], pt1[:])
                                emit_proj(1)

                            # tail: denominators, reciprocal, broadcast, scale
                            ps_d0 = pax.tile([1, 512], F32, tag="x")
                            nc.tensor.matmul(ps_d0[:], onesc_sb[:], d0[:],
                                             start=True, stop=True)
                            ps_d1 = pax.tile([1, 512], F32, tag="x")
                            nc.tensor.matmul(ps_d1[:], onesc_sb[:], d1[:],
                                             start=True, stop=True)
                            emit_proj(1)
                            rden0 = rtp.tile([1, 512], F32, tag="rf")
                            nc.vector.reciprocal_approx_fast(rden0[:],
                                                             ps_d0[:])
                            rden1 = rtp.tile([1, 512], F32, tag="rf")
                            nc.vector.reciprocal_approx_fast(rden1[:],
                                                             ps_d1[:])
                            rdenb0 = rtp.tile([1, 512], BF16, tag="rc")
                            nc.vector.tensor_copy(rdenb0[:], rden0[:])
                            rdenb1 = rtp.tile([1, 512], BF16, tag="rc")
                            nc.vector.tensor_copy(rdenb1[:], rden1[:])
                            ps_bc0 = pax.tile([128, 512], F32, tag="x")
                            nc.tensor.matmul(ps_bc0[:], onesr_sb[:],
                                             rdenb0[:],
                                             start=True, stop=True)
                            emit_proj(1)
                            rb0 = rtp.tile([128, 512], BF16, tag="rb")
                            nc.vector.tensor_copy(rb0[:], ps_bc0[:])
                            nc.vector.tensor_mul(yT[:, h0, t0g:t0g + 512],
                                                 ps_o0[:], rb0[:])
                            ps_bc1 = pax.tile([128, 512], F32, tag="x")
                            nc.tensor.matmul(ps_bc1[:], onesr_sb[:],
                                             rdenb1[:],
                                             start=True, stop=True)
                            emit_proj(1)
                            rb1 = rtp.tile([128, 512], BF16, tag="rb")
                            nc.vector.tensor_copy(rb1[:], ps_bc1[:])
                            nc.vector.tensor_mul(yT[:, h1, t0g:t0g + 512],
                                                 ps_o1[:], rb1[:])
                            emit_proj(1)

                        # all 4 heads of this (b, tcq) done: release proj rows
                        for cc in range(C // 512):
                            for ti in range(4):
                                proj_queue.append((t0g + ti * 128, cc))

                # drain remaining proj tiles
                emit_proj(len(proj_queue))

                for c in reversed(psum_ctx):
                    c.__exit__(None, None, None)
    nc.compile()
    return nc


def _prep_inputs(x, cos, sin, Wqkv, Wproj):
    bf = ml_dtypes.bfloat16
    xTn = np.ascontiguousarray(x.reshape(TOK, C).T).astype(bf)
    cosf = np.ascontiguousarray(cos.T.astype(np.float32))        # [128, T]
    sinT = sin.T.astype(np.float32)
    sinb = np.concatenate([-sinT[0:64], sinT[64:128]], axis=0)
    sinb = np.ascontiguousarray(sinb, dtype=np.float32)
    p = np.arange(128)[:, None, None]
    j = np.arange(4)[None, :, None]
    f = np.arange(512)[None, None, :]
    masks = ((j * 128 + p) <= f).astype(bf)
    onesc = np.ones([128, 1], dtype=bf)
    onesr = np.ones([1, 128], dtype=bf)
    ident = np.eye(128, dtype=np.float32).astype(bf)
    in_maps = []
    for g in range(N_CORES):
        in_maps.append({
            "xt": xTn,
            "wqkv": np.ascontiguousarray(Wqkv[:, g * GCOLS:(g + 1) * GCOLS]).astype(bf),
            "wproj": np.ascontiguousarray(Wproj[g * 512:(g + 1) * 512, :]).astype(bf),
            "cosf": cosf, "sinb": sinb, "masks": masks,
            "onesc": onesc, "onesr": onesr, "ident": ident,
        })
    return in_maps


def kernel(x, cos, sin, Wqkv, Wproj, _trace=False):
    global _NC_CACHE
    x = np.asarray(x, dtype=np.float32)
    cos = np.asarray(cos, dtype=np.float32)
    sin = np.asarray(sin, dtype=np.float32)
    Wqkv = np.asarray(Wqkv, dtype=np.float32)
    Wproj = np.asarray(Wproj, dtype=np.float32)
    if _NC_CACHE is None:
        _NC_CACHE = build_nc()
    nc = _NC_CACHE
    in_maps = _prep_inputs(x, cos, sin, Wqkv, Wproj)
    res = run_bass_kernel_spmd(nc, in_maps, core_ids=list(range(N_CORES)),
                               trace=_trace)
    acc = np.zeros([TOK, C], dtype=np.float32)
    for r in res.results:
        acc += r["out"].astype(np.float32)
    if _trace:
        kernel._last_exec_ns = res.exec_time_ns
        kernel._last_trace = res.instructions_and_trace
    return acc.reshape(B, T, C)


# revision 7
# speedup vs baseline: 1.1857x; 1.0504x over previous
"""Causal GQA self-attention (B=2,T=2048,C=4096, 32 q-heads, 8 kv-groups, hs=128)
sharded tensor-parallel across 8 TRN2 NeuronCores: one kv-group (4 q heads) per core.

Per-core pipeline (all activations feature-major, i.e. transposed):
  1. qkvT = Wqkv_g^T @ x^T           (bf16 matmul, fp32 psum)
     RoPE applied to q/k in fp32 during psum->sbuf epilogue, cast bf16
  2. v transposed to token-major via PE transpose
  3. attention per (batch, q-chunk, head-pair): S^T tiles (kv on partitions,
     q on free dim), exp via ACT with diagonal-trimmed columns, causal mask
     by multiply on diagonal tiles, O^T accumulated via v_tok^T @ P^T.
     Softmax denominator: exp tiles accumulated on DVE into dsum, one
     M=1 matmul per head contracts partitions; reciprocal + PE outer-product
     broadcast (bf16) + multiply into O^T.
     Head pairs share PE weight loads (same kT / v_tok stationary operand).
  4. proj tiles (y^T slices^T @ Wproj_g) interleaved into the attention
     stream as PE filler, so exp latency hides under proj matmuls.
Host sums the 8 partial outputs in fp32.
"""
import math

import numpy as np
import ml_dtypes

import concourse.bass as bass
import concourse.mybir as mybir
import concourse.tile as tile
from concourse import bacc
from concourse.bass_utils import run_bass_kernel_spmd

BF16 = mybir.dt.bfloat16
F32 = mybir.dt.float32
AF = mybir.ActivationFunctionType

N_CORES = 8
B, T, C = 2, 2048, 4096
HS = 128
QPK = 4              # q heads per kv group
GCOLS = (QPK + 2) * HS   # 768 qkv columns per group
TOK = B * T          # 4096
SCALE = float(1.0 / np.sqrt(np.float32(HS)))

_NC_CACHE = None


def build_nc():
    nc = bacc.Bacc("TRN2", target_bir_lowering=False, debug=False,
                   num_devices=N_CORES)
    xT = nc.dram_tensor("xt", [C, TOK], BF16, kind="ExternalInput").ap()
    wqkv = nc.dram_tensor("wqkv", [C, GCOLS], BF16, kind="ExternalInput").ap()
    wproj = nc.dram_tensor("wproj", [QPK * HS, C], BF16, kind="ExternalInput").ap()
    cosf = nc.dram_tensor("cosf", [128, T], F32, kind="ExternalInput").ap()
    sinb = nc.dram_tensor("sinb", [128, T], F32, kind="ExternalInput").ap()
    masks = nc.dram_tensor("masks", [128, 4, 512], BF16, kind="ExternalInput").ap()
    onesc = nc.dram_tensor("onesc", [128, 1], BF16, kind="ExternalInput").ap()
    onesr = nc.dram_tensor("onesr", [1, 128], BF16, kind="ExternalInput").ap()
    ident = nc.dram_tensor("ident", [128, 128], BF16, kind="ExternalInput").ap()
    out = nc.dram_tensor("out", [TOK, C], BF16, kind="ExternalOutput").ap()

    xT_r = xT.rearrange("(ko p) t -> p ko t", p=128)        # [128, 32, TOK]
    wqkv_r = wqkv.rearrange("(ko p) m -> p ko m", p=128)    # [128, 32, 768]
    wproj_r = wproj.rearrange("(h p) c -> p h c", p=128)    # [128, 4, C]

    with tile.TileContext(nc) as tc:
        with tc.tile_pool(name="mid", bufs=1) as mid:
            qT = mid.tile([128, QPK, TOK], BF16)   # rope'd q, feature-major
            kT = mid.tile([128, TOK], BF16)
            vT = mid.tile([128, TOK], BF16)

            # ---------------- stage 1: qkv + rope ----------------
            with tc.tile_pool(name="s1c", bufs=1) as s1c, \
                 tc.tile_pool(name="s1x", bufs=4) as s1x, \
                 tc.tile_pool(name="s1r", bufs=3) as s1r, \
                 tc.tile_pool(name="ps1", bufs=4, space="PSUM") as ps1:
                wq_sb = s1c.tile([128, 32, GCOLS], BF16)
                # first-chunk weight columns (m 0-2) + first x chunk first, so
                # the first matmul chain starts as early as DMA bandwidth allows
                for kk in range(32):
                    nc.sync.dma_start(wq_sb[:, kk, 0:384], wqkv_r[:, kk, 0:384])
                cos_sb = s1c.tile([128, T], F32)
                sin_sb = s1c.tile([128, T], F32)
                x0 = []
                for xh in range(2):
                    xt_t = s1x.tile([128, 16, 512], BF16, tag="x", name="x0")
                    for q4 in range(4):
                        nc.sync.dma_start(
                            xt_t[:, q4 * 4:(q4 + 1) * 4, :],
                            xT_r[:, xh * 16 + q4 * 4:xh * 16 + (q4 + 1) * 4,
                                 0:512])
                    x0.append(xt_t)
                nc.sync.dma_start(cos_sb[:], cosf[:])
                nc.sync.dma_start(sin_sb[:], sinb[:])
                for kk in range(32):
                    nc.sync.dma_start(wq_sb[:, kk, 384:GCOLS],
                                      wqkv_r[:, kk, 384:GCOLS])

                for tcch in range(TOK // 512):       # 8 chunks of 512 tokens
                    t0 = tcch * 512
                    tb = t0 % T                      # position within batch
                    if tcch == 0:
                        xa, xb = x0
                    else:
                        xa = s1x.tile([128, 16, 512], BF16, tag="x")
                        for q4 in range(4):
                            nc.sync.dma_start(
                                xa[:, q4 * 4:(q4 + 1) * 4, :],
                                xT_r[:, q4 * 4:(q4 + 1) * 4, t0:t0 + 512])
                        xb = s1x.tile([128, 16, 512], BF16, tag="x")
                        for q4 in range(4):
                            nc.sync.dma_start(
                                xb[:, q4 * 4:(q4 + 1) * 4, :],
                                xT_r[:, 16 + q4 * 4:16 + (q4 + 1) * 4,
                                     t0:t0 + 512])
                    for m in range(6):
                        ps = ps1.tile([128, 512], F32, tag="ps")
                        for kh, xt_t in ((0, xa), (1, xb)):
                            for ki in range(16):
                                kk = kh * 16 + ki
                                nc.tensor.matmul(
                                    ps[:], wq_sb[:, kk, m * 128:(m + 1) * 128],
                                    xt_t[:, ki, :],
                                    start=(kk == 0), stop=(kk == 31))
                        if m == 5:                   # v: plain copy+cast
                            nc.vector.tensor_copy(vT[:, t0:t0 + 512], ps[:])
                        else:                        # q heads 0-3 / k: rope
                            t1 = s1r.tile([128, 512], F32, tag="t1")
                            nc.vector.tensor_mul(t1[:], ps[:],
                                                 cos_sb[:, tb:tb + 512])
                            u = s1r.tile([128, 512], F32, tag="u")
                            nc.vector.tensor_mul(u[0:64, :], ps[64:128, :],
                                                 sin_sb[0:64, tb:tb + 512])
                            nc.vector.tensor_mul(u[64:128, :], ps[0:64, :],
                                                 sin_sb[64:128, tb:tb + 512])
                            if m < 4:
                                dst = qT[:, m, t0:t0 + 512]
                            else:
                                dst = kT[:, t0:t0 + 512]
                            nc.vector.tensor_add(dst, t1[:], u[:])

            # ---------------- stages 2-4: attention + proj ----------------
            with tc.tile_pool(name="s3c", bufs=1) as s3c, \
                 tc.tile_pool(name="s3p", bufs=8) as s3p, \
                 tc.tile_pool(name="s3d", bufs=2) as s3d, \
                 tc.tile_pool(name="rtp", bufs=2) as rtp, \
                 tc.tile_pool(name="s4o", bufs=4) as s4o:
                ident_sb = s3c.tile([128, 128], BF16)
                nc.sync.dma_start(ident_sb[:], ident[:])
                onesc_sb = s3c.tile([128, 1], BF16)
                nc.sync.dma_start(onesc_sb[:], onesc[:])
                onesr_sb = s3c.tile([1, 128], BF16)
                nc.sync.dma_start(onesr_sb[:], onesr[:])
                masks_sb = s3c.tile([128, 4, 512], BF16)
                nc.sync.dma_start(masks_sb[:], masks[:])
                v_tok = s3c.tile([128, 32, 128], BF16)
                yT = s3c.tile([128, QPK, TOK], BF16)
                wp_sb = s3c.tile([128, 4, C], BF16)
                for hh in range(4):
                    nc.sync.dma_start(wp_sb[:, hh, :], wproj_r[:, hh, :])

                with tc.tile_pool(name="psT", bufs=4, space="PSUM") as psT:
                    # v -> token-major via PE transpose
                    for si in range(32):
                        pt_ps = psT.tile([128, 128], BF16, tag="tp")
                        nc.tensor.transpose(pt_ps[:],
                                            vT[:, si * 128:(si + 1) * 128],
                                            ident_sb[:])
                        nc.vector.tensor_copy(v_tok[:, si, :], pt_ps[:])

                psum_ctx = [tc.tile_pool(name="pax", bufs=3, space="PSUM"),
                            tc.tile_pool(name="pso", bufs=3, space="PSUM"),
                            tc.tile_pool(name="ps4", bufs=2, space="PSUM")]
                pax, pso, ps4 = [c.__enter__() for c in psum_ctx]

                # proj work queue: tiles become available per (b, tcq) unit
                proj_queue = []     # (t0 row, cc) pairs ready to emit
                proj_state = {"n": 0}

                def emit_proj(k):
                    for _ in range(k):
                        if not proj_queue:
                            return
                        pt0r, cc = proj_queue.pop(0)
                        ps_p = ps4.tile([128, 512], F32, tag="p")
                        for h in range(QPK):
                            nc.tensor.matmul(
                                ps_p[:], yT[:, h, pt0r:pt0r + 128],
                                wp_sb[:, h, cc * 512:(cc + 1) * 512],
                                start=(h == 0), stop=(h == 3))
                        ob = s4o.tile([128, 512], BF16, tag="ob")
                        if proj_state["n"] % 2 == 0:
                            nc.vector.tensor_copy(ob[:], ps_p[:])
                        else:
                            nc.scalar.copy(ob[:], ps_p[:])
                        proj_state["n"] += 1
                        nc.sync.dma_start(
                            out[pt0r:pt0r + 128, cc * 512:(cc + 1) * 512],
                            ob[:])

                # zero-init the pt rotation buffers once: diagonal-trimmed exp
                # leaves a column prefix unwritten that must multiply to 0
                for _ in range(8):
                    ptz = s3p.tile([128, 512], BF16, tag="pt")
                    nc.vector.memset(ptz[:], 0.0)

                for b in range(B):
                    for tcq in range(4):              # 512-token q chunks
                        t0g = b * T + tcq * 512
                        n_s = (tcq + 1) * 4
                        for hp in range(2):           # head pairs
                            h0, h1 = 2 * hp, 2 * hp + 1
                            ps_o0 = pso.tile([128, 512], F32, tag="o")
                            ps_o1 = pso.tile([128, 512], F32, tag="o")
                            d0 = d1 = None
                            for si in range(n_s):
                                s0g = b * T + si * 128
                                j = si - 4 * tcq
                                lo = 128 * max(j, 0)   # diagonal column trim
                                ps_s0 = pax.tile([128, 512], F32, tag="x")
                                nc.tensor.matmul(
                                    ps_s0[:, lo:], kT[:, s0g:s0g + 128],
                                    qT[:, h0, t0g + lo:t0g + 512],
                                    start=True, stop=True)
                                ps_s1 = pax.tile([128, 512], F32, tag="x")
                                nc.tensor.matmul(
                                    ps_s1[:, lo:], kT[:, s0g:s0g + 128],
                                    qT[:, h1, t0g + lo:t0g + 512],
                                    start=True, stop=True)
                                emit_proj(1)
                                pt0 = s3p.tile([128, 512], BF16, tag="pt")
                                nc.scalar.activation(pt0[:, lo:], ps_s0[:, lo:],
                                                     AF.Exp, scale=SCALE)
                                pt1 = s3p.tile([128, 512], BF16, tag="pt")
                                nc.scalar.activation(pt1[:, lo:], ps_s1[:, lo:],
                                                     AF.Exp, scale=SCALE)
                                if j >= 0:
                                    nc.vector.tensor_mul(pt0[:], pt0[:],
                                                         masks_sb[:, j, :])
                                    nc.vector.tensor_mul(pt1[:], pt1[:],
                                                         masks_sb[:, j, :])
                                nc.tensor.matmul(
                                    ps_o0[:], v_tok[:, b * 16 + si, :], pt0[:],
                                    start=(si == 0), stop=(si == n_s - 1))
                                nc.tensor.matmul(
                                    ps_o1[:], v_tok[:, b * 16 + si, :], pt1[:],
                                    start=(si == 0), stop=(si == n_s - 1))
                                if si == 0:
                                    d0 = s3d.tile([128, 512], BF16, tag="d0")
                                    nc.vector.tensor_copy(d0[:], pt0[:])
                                    d1 = s3d.tile([128, 512], BF16, tag="d1")
                                    nc.vector.tensor_copy(d1[:], pt1[:])
                                else:
                                    nc.vector.tensor_add(d0[:], d0[:], pt0[:])
                                    nc.vector.tensor_add(d1[:], d1[:], pt1[:])

                            # tail: denominators, reciprocal, broadcast, scale
                            ps_d0 = pax.tile([1, 512], F32, tag="x")
                            nc.tensor.matmul(ps_d0[:], onesc_sb[:], d0[:],
                                             start=True, stop=True)
                            ps_d1 = pax.tile([1, 512], F32, tag="x")
                            nc.tensor.matmul(ps_d1[:], onesc_sb[:], d1[:],
                                             start=True, stop=True)
                            emit_proj(2)
                            rden0 = rtp.tile([1, 512], F32, tag="rf")
                            nc.vector.reciprocal_approx_fast(rden0[:],
                                                             ps_d0[:])
                            rden1 = rtp.tile([1, 512], F32, tag="rf")
                            nc.vector.reciprocal_approx_fast(rden1[:],
                                                             ps_d1[:])
                            rdenb0 = rtp.tile([1, 512], BF16, tag="rc")
                            nc.vector.tensor_copy(rdenb0[:], rden0[:])
                            rdenb1 = rtp.tile([1, 512], BF16, tag="rc")
                            nc.vector.tensor_copy(rdenb1[:], rden1[:])
                            ps_bc0 = pax.tile([128, 512], F32, tag="x")
                            nc.tensor.matmul(ps_bc0[:], onesr_sb[:],
                                             rdenb0[:],
                                             start=True, stop=True)
                            emit_proj(1)
                            rb0 = rtp.tile([128, 512], BF16, tag="rb")
                            nc.vector.tensor_copy(rb0[:], ps_bc0[:])
                            nc.vector.tensor_mul(yT[:, h0, t0g:t0g + 512],
                                                 ps_o0[:], rb0[:])
                            ps_bc1 = pax.tile([128, 512], F32, tag="x")
                            nc.tensor.matmul(ps_bc1[:], onesr_sb[:],
                                             rdenb1[:],
                                             start=True, stop=True)
                            emit_proj(1)
                            rb1 = rtp.tile([128, 512], BF16, tag="rb")
                            nc.vector.tensor_copy(rb1[:], ps_bc1[:])
                            nc.vector.tensor_mul(yT[:, h1, t0g:t0g + 512],
                                                 ps_o1[:], rb1[:])
                            emit_proj(2)

                        # all 4 heads of this (b, tcq) done: release proj rows
                        for cc in range(C // 512):
                            for ti in range(4):
                                proj_queue.append((t0g + ti * 128, cc))

                # drain remaining proj tiles
                emit_proj(len(proj_queue))

                for c in reversed(psum_ctx):
                    c.__exit__(None, None, None)
    nc.compile()
    return nc


def _prep_inputs(x, cos, sin, Wqkv, Wproj):
    bf = ml_dtypes.bfloat16
    xTn = np.ascontiguousarray(x.reshape(TOK, C).T).astype(bf)
    cosf = np.ascontiguousarray(cos.T.astype(np.float32))        # [128, T]
    sinT = sin.T.astype(np.float32)
    sinb = np.concatenate([-sinT[0:64], sinT[64:128]], axis=0)
    sinb = np.ascontiguousarray(sinb, dtype=np.float32)
    p = np.arange(128)[:, None, None]
    j = np.arange(4)[None, :, None]
    f = np.arange(512)[None, None, :]
    masks = ((j * 128 + p) <= f).astype(bf)
    onesc = np.ones([128, 1], dtype=bf)
    onesr = np.ones([1, 128], dtype=bf)
    ident = np.eye(128, dtype=np.float32).astype(bf)
    in_maps = []
    for g in range(N_CORES):
        in_maps.append({
            "xt": xTn,
            "wqkv": np.ascontiguousarray(Wqkv[:, g * GCOLS:(g + 1) * GCOLS]).astype(bf),
            "wproj": np.ascontiguousarray(Wproj[g * 512:(g + 1) * 512, :]).astype(bf),
            "cosf": cosf, "sinb": sinb, "masks": masks,
            "onesc": onesc, "onesr": onesr, "ident": ident,
        })
    return in_maps


def kernel(x, cos, sin, Wqkv, Wproj, _trace=False):
    global _NC_CACHE
    x = np.asarray(x, dtype=np.float32)
    cos = np.asarray(cos, dtype=np.float32)
    sin = np.asarray(sin, dtype=np.float32)
    Wqkv = np.asarray(Wqkv, dtype=np.float32)
    Wproj = np.asarray(Wproj, dtype=np.float32)
    if _NC_CACHE is None:
        _NC_CACHE = build_nc()
    nc = _NC_CACHE
    in_maps = _prep_inputs(x, cos, sin, Wqkv, Wproj)
    res = run_bass_kernel_spmd(nc, in_maps, core_ids=list(range(N_CORES)),
                               trace=_trace)
    acc = np.zeros([TOK, C], dtype=np.float32)
    for r in res.results:
        acc += r["out"].astype(np.float32)
    if _trace:
        kernel._last_exec_ns = res.exec_time_ns
        kernel._last_trace = res.instructions_and_trace
    return acc.reshape(B, T, C)
